# revision 65
# baseline (speedup 1.0000x reference)
"""Trainium2 Bass kernel for nn_BlockDiagonalLinear_text (hyperbolic block-diag linear).

Math: the reference's per-row operations are all scalar row-scalings, so
  out = alpha_row * y   with  y = x @ blockdiag(W_1..W_16).T
where alpha_row is a chain of tanh/artanh/sqrt scalars of ||x_row|| and
||y_row||.  Both artanh(clip(tanh(t))) compositions in that chain are
min(t, artanh(clip)) up to fp32 roundoff (validated to 3e-4 rel), which
removes two Ln activations and ~10 small ops per chain.

Sharding: data-parallel over rows. 8192 rows -> 8 cores x 1024 rows.
Weights replicated. bf16 on the whole matmul path (l2 tolerance is 2e-2;
bf16 lands ~2e-3): inputs are cast to bf16 on host, output is written
bf16 and cast back to f32 on host, halving all DMA streams.

Each core's x shard is laid out on host in transposed (k-on-partition)
tile order -- xt[tile][p, kc*128+r] = x[tile*128+r, kc*128+p] -- the
layout the PE wants for the stationary operand. This removes all
on-device transposes and their PSUM->SBUF copies. qx = ||x_row||^2 is
computed with a DVE square of xt (2-byte fast path) plus per-k-chunk
matmuls against a ones column (stationary = xsq chunk), accumulating
over the 32 chunks in PSUM; the result lands directly on partitions.

Per-core per 128-row tile:
  DMA xt tile -> DVE xt^2 -> PE qx-matmuls + block matmuls -> Pool
  evacuates y banks (f32 PSUM -> bf16 SBUF) -> ACT y^2 row-sums (qy).
  Every CHAIN_K tiles a batched scalar chain ([128,K] ops on Pool, DVE
  recips, ACT Ln/Exp) produces alpha; chain emission is delayed one
  tile so its serial span never head-of-line-blocks Pool's y-copies.
  DVE scales y (2-byte fast path); outs ship as one batched DMA per
  chain group.

All DMAs ride the SP queue in an order where x-ins carry no waits
(one pool slot per tile) and outs trail, so out-DMAs waiting on scale
results never block the input stream. An explicit LoadActFuncSet pins
the natural_log_exp table (square, ln, exp in one table) so the bacc
table-load pass doesn't thrash tables.
"""
import sys
import numpy as np

for _p in ("/opt/trn_rl_repo", "/root/.axon_site/_ro/trn_rl_repo"):
    if _p not in sys.path:
        sys.path.append(_p)

import ml_dtypes
import concourse.bass as bass
import concourse.bacc as bacc
import concourse.mybir as mybir
from concourse import tile
from concourse.bass_utils import run_bass_kernel_spmd

R, BS = 16, 256           # 16 diagonal blocks of 256x256
D = R * BS                # 4096
P = 128                   # partitions
N_CORES = 8
ROWS_TOTAL = 4 * 2048     # 8192
ROWS_CORE = ROWS_TOTAL // N_CORES   # 1024
NT = ROWS_CORE // P       # 8 tiles of 128 rows per core
CHAIN_K = 1               # tiles per batched scalar chain
WCOLS = 2 * R * BS        # 8192 weight columns (k-major chunks)

f32 = mybir.dt.float32
bf16 = mybir.dt.bfloat16
AF = mybir.ActivationFunctionType
OP = mybir.AluOpType

CLIP_Z = float(np.float32(1.0) - np.float32(1e-5))          # 0.99999
MAXNORM = float(np.float32(1.0 - 1e-3) / np.float32(0.1))   # 9.99
ATC1_X2 = float(2.0 * np.arctanh(np.float64(np.float32(CLIP_Z))))
ATC2 = float(np.arctanh(0.999))

# act_info.json table index of natural_log_exp_and_others (square, ln,
# exp, copy, identity in one table).
NLE_TABLE_ID = 6

XB = D // P               # 32 k-chunks per tile
YB = 512                  # y bank width (f32): [128, 512] = one bank
NBANK_Y = D // YB         # 8 y banks per tile


def build_nc():
    nc = bacc.Bacc()
    xt_d = nc.declare_dram_parameter("xt", [ROWS_CORE, D], bf16, isOutput=False)
    w_d = nc.declare_dram_parameter("w", [P, WCOLS], bf16, isOutput=False)
    out_d = nc.declare_dram_parameter("out", [ROWS_CORE, D], bf16, isOutput=True)

    with tile.TileContext(nc) as tc:
        with (
            tc.tile_pool(name="wpool", bufs=1) as wpool,
            tc.tile_pool(name="xpool", bufs=NT) as xpool,
            tc.tile_pool(name="sqpool", bufs=2) as sqpool,
            tc.tile_pool(name="ypool", bufs=6) as ypool,
            tc.tile_pool(name="opool", bufs=2) as opool,
            tc.tile_pool(name="scrpool", bufs=1) as scrpool,
            tc.tile_pool(name="stats", bufs=2) as stats,
            tc.tile_pool(name="psq", bufs=2, space="PSUM") as psq,
            tc.tile_pool(name="psy", bufs=3, space="PSUM") as psy,
        ):
            # SP DMA order: all 8 xt tiles (no waits - one slot each)
            # interleaved with weight quarters; outs are emitted later so
            # they trail in queue order.
            w_sb = wpool.tile([P, WCOLS], bf16, name="w_sb")
            x_tiles = []
            wq = WCOLS // 4
            for i in range(NT):
                x_sb = xpool.tile([P, D], bf16, tag="x", name=f"x_{i}")
                if i == 0:
                    # first tile split in halves: PE can start on the low
                    # k-chunks ~1.5us earlier
                    nc.sync.dma_start(out=x_sb[:, 0:D // 2],
                                      in_=xt_d[0:P, 0:D // 2])
                    nc.sync.dma_start(out=x_sb[:, D // 2:D],
                                      in_=xt_d[0:P, D // 2:D])
                else:
                    nc.sync.dma_start(out=x_sb[:],
                                      in_=xt_d[i * P:(i + 1) * P, :])
                x_tiles.append(x_sb)
                if i == 0:
                    for q in range(4):
                        nc.sync.dma_start(out=w_sb[:, q * wq:(q + 1) * wq],
                                          in_=w_d[:, q * wq:(q + 1) * wq])
            scratch = scrpool.tile([P, D], bf16, name="scratch")
            ones_full = scrpool.tile([P, 2], bf16, name="ones_full")
            nc.vector.memzero(ones_full[:])
            nc.gpsimd.tensor_scalar_add(ones_full[:], ones_full[:], 1.0)
            ones_sb = ones_full[:, 0:1]

            # Pin the one act table that covers Square/Ln/Exp so the bacc
            # load pass doesn't alternate natural_log <-> exp tables.
            ld = mybir.InstLoadActFuncSet(
                name=nc.get_next_instruction_name(), ins=[], outs=[],
                act_func_set_id=NLE_TABLE_ID)
            nc.scalar.add_instruction(ld)

            def st(shape, tag, dtype=f32):
                return stats.tile(shape, dtype, tag=tag, name=tag)

            V = nc.vector
            G = nc.gpsimd
            K = CHAIN_K

            def emit_tile(i, qq, j):
                """qx + block matmuls + y evacuation + qy for tile i"""
                xt_sb = x_tiles[i]

                # block matmuls: y[:, r*256:+256] = x_blk_r @ W_r.T
                # grouped two blocks per PSUM bank; Pool evacuates each
                # bank to bf16 SBUF
                y_sb = ypool.tile([P, D], bf16, tag="y", name=f"y_{i}")
                for g in range(NBANK_Y // 2):
                    py = psy.tile([P, 2 * YB], f32, tag="py",
                                  name=f"py_{i}_{g}")
                    for rr in range(4):
                        r = 4 * g + rr
                        for c in range(2):
                            kc = 2 * r + c
                            nc.tensor.matmul(
                                py[:, rr * BS:(rr + 1) * BS],
                                xt_sb[:, kc * P:(kc + 1) * P],
                                w_sb[:, kc * BS:(kc + 1) * BS],
                                start=(c == 0), stop=(c == 1),
                            )
                    # PSUM evacuation: GPSIMD can't touch PSUM on real
                    # HW, so split double-bank copies between ACT and DVE
                    dstv = y_sb[:, g * 2 * YB:(g + 1) * 2 * YB]
                    if g % 2 == 0:
                        nc.scalar.activation(dstv, py[:], AF.Copy)
                    else:
                        V.tensor_copy(dstv, py[:])

                # qx = sum_k x^2: DVE squares the transposed tile (2-byte
                # fast path); per-chunk matmuls with stationary = xsq
                # chunk and moving = ones column accumulate the partition
                # sums in PSUM, landing qx per-row on partitions.
                xsq = sqpool.tile([P, D], bf16, tag="xsq", name=f"xsq_{i}")
                V.tensor_tensor(out=xsq[:], in0=xt_sb[:], in1=xt_sb[:],
                                op=OP.mult)
                pqx = psq.tile([P, YB], f32, tag="pqx", name=f"pqx_{i}")
                for kc in range(XB):
                    nc.tensor.matmul(
                        pqx[0:P, 0:1], xsq[:, kc * P:(kc + 1) * P], ones_sb,
                        start=(kc == 0), stop=(kc == XB - 1),
                    )
                V.tensor_copy(qq[:, j:j + 1], pqx[0:P, 0:1])

                # qy = sum_j y^2 (row-wise) on ACT
                nc.scalar.activation(scratch[:], y_sb[:], AF.Square,
                                     accum_out=qq[:, K + j:K + j + 1])
                return y_sb

            def emit_chain(b, qq, y_tiles):
                """batched per-row scalar chain + scale + batched out-DMA.
                artanh(clip(tanh(t))) folds to min(t, artanh(clip))."""
                lnq = st([P, 2 * K], "lnq")
                nc.scalar.activation(lnq[:], qq[:], AF.Ln)
                U = st([P, 2 * K], "U")   # [u | y_n] = exp(0.5 ln q)
                nc.scalar.activation(U[:], lnq[:], AF.Exp, scale=0.5)

                t1 = st([P, K], "t1")     # 0.1 * max(u, 1e-5)
                V.tensor_scalar(out=t1[:], in0=U[:, 0:K], scalar1=1e-5,
                                scalar2=0.1, op0=OP.max, op1=OP.mult)
                r1 = st([P, K], "r1")
                V.reciprocal(r1[:], t1[:])
                d_ = st([P, K], "d_")     # 2*artanh(clip(tanh(t1)))
                V.tensor_scalar(out=d_[:], in0=t1[:], scalar1=2.0,
                                scalar2=ATC1_X2, op0=OP.mult, op1=OP.min)
                yns = st([P, K], "yns")
                V.tensor_scalar_max(yns[:], U[:, K:2 * K], 1e-20)
                w1 = st([P, K], "w1")
                V.tensor_mul(w1[:], U[:, K:2 * K], r1[:])
                w2 = st([P, K], "w2")
                V.tensor_mul(w2[:], w1[:], d_[:])
                argt = st([P, K], "argt")
                V.tensor_scalar(out=argt[:], in0=w2[:], scalar1=0.05,
                                scalar2=15.0, op0=OP.mult, op1=OP.min)
                Et = st([P, K], "Et")
                nc.scalar.activation(Et[:], argt[:], AF.Exp, scale=2.0)
                e2 = st([P, K], "e2")
                V.tensor_scalar_add(e2[:], Et[:], 1.0)
                r3 = st([P, K], "r3")
                V.reciprocal(r3[:], e2[:])
                ttx = st([P, K], "ttx")   # tanh(arg_t)
                V.tensor_scalar(out=ttx[:], in0=r3[:], scalar1=-2.0, scalar2=1.0,
                                op0=OP.mult, op1=OP.add)
                nrm = st([P, K], "nrm")
                V.tensor_scalar(out=nrm[:], in0=ttx[:], scalar1=10.0,
                                scalar2=1e-5, op0=OP.mult, op1=OP.max)
                ryn = st([P, K], "ryn")
                V.reciprocal(ryn[:], yns[:])
                gs = st([P, K], "gs")
                V.tensor_mul(gs[:], ttx[:], ryn[:])
                rn = st([P, K], "rn")
                V.reciprocal(rn[:], nrm[:])
                pf = st([P, K], "pf")     # min(MAXNORM/nrm, 1)
                V.tensor_scalar(out=pf[:], in0=rn[:], scalar1=MAXNORM,
                                scalar2=1.0, op0=OP.mult, op1=OP.min)
                zb = st([P, K], "zb")     # 0.1*min(max(10 ttx,1e-5),MAXNORM)
                V.tensor_scalar(out=zb[:], in0=ttx[:], scalar1=1e-6,
                                scalar2=0.999, op0=OP.max, op1=OP.min)
                db = st([P, K], "db")     # 2*artanh(clip(tanh(argt)))
                V.tensor_scalar(out=db[:], in0=argt[:], scalar1=ATC2,
                                scalar2=2.0, op0=OP.min, op1=OP.mult)
                rzb = st([P, K], "rzb")
                V.reciprocal(rzb[:], zb[:])
                a1 = st([P, K], "a1")
                V.tensor_mul(a1[:], gs[:], pf[:])
                a2 = st([P, K], "a2")
                V.tensor_mul(a2[:], db[:], rzb[:])
                al = st([P, K], "al")
                V.tensor_mul(al[:], a1[:], a2[:])
                mask = st([P, K], "mask")
                V.tensor_scalar(out=mask[:], in0=qq[:, K:2 * K], scalar1=0.0,
                                scalar2=None, op0=OP.is_gt)
                alm = st([P, K], "alm")
                V.tensor_mul(alm[:], al[:], mask[:])

                # out = y * alpha * 5  (5 = 10 from gs x 0.5 from artanh
                # halves); both tiles staged into one buffer, shipped as a
                # single batched DMA trailing on the SP queue
                y_out = opool.tile([P, K * D], bf16, tag="yo", name=f"yo_{b}")
                for j, y_sb in enumerate(y_tiles):
                    V.tensor_scalar(out=y_out[:, j * D:(j + 1) * D],
                                    in0=y_sb[:],
                                    scalar1=alm[:, j:j + 1], scalar2=5.0,
                                    op0=OP.mult, op1=OP.mult)
                i0 = b * K
                dst = out_d[i0 * P:(i0 + K) * P, :].rearrange(
                    "(h p) c -> p h c", p=P)
                nc.sync.dma_start(
                    out=dst, in_=y_out[:].rearrange("p (h c) -> p h c", h=K))

            # main loop: chain emitted right after its batch's tiles
            qq = None
            y_tiles = []
            for i in range(NT):
                b, j = divmod(i, K)
                if j == 0:
                    qq = st([P, 2 * K], "qq")
                    y_tiles = []
                y_tiles.append(emit_tile(i, qq, j))
                if j == K - 1:
                    emit_chain(b, qq, y_tiles)
    nc.finalize()
    return nc


_NC = None


def _get_nc():
    global _NC
    if _NC is None:
        _NC = build_nc()
    return _NC


def _prep_weights(weights: np.ndarray) -> np.ndarray:
    # w_sb[:, (2r+c)*256:+256][p, j] = W[r, j, k=c*128+p]
    wt = (weights.astype(np.float32).transpose(0, 2, 1)      # [r, k, j]
          .reshape(R, 2, P, BS).transpose(2, 0, 1, 3)        # [p, r, c, j]
          .reshape(P, WCOLS))
    return np.ascontiguousarray(wt.astype(ml_dtypes.bfloat16))


def _prep_x(x: np.ndarray) -> np.ndarray:
    # Transposed shard layout: per core, per 128-row tile,
    # xt[p, kc*128 + r] = x[tile*128 + r, kc*128 + p]
    xf = np.asarray(x, dtype=np.float32).reshape(
        N_CORES, NT, P, XB, P)                  # [core, tile, r, kc, p]
    xt = xf.transpose(0, 1, 4, 3, 2)            # [core, tile, p, kc, r]
    return np.ascontiguousarray(xt).astype(
        ml_dtypes.bfloat16).reshape(N_CORES, ROWS_CORE, D)


def kernel(x: np.ndarray, weights: np.ndarray) -> np.ndarray:
    nc = _get_nc()
    xt = _prep_x(x)
    wid = _prep_weights(np.asarray(weights))
    in_maps = [
        {"xt": xt[i], "w": wid}
        for i in range(N_CORES)
    ]
    res = run_bass_kernel_spmd(nc, in_maps, list(range(N_CORES)))
    out = np.concatenate([res.results[i]["out"] for i in range(N_CORES)], axis=0)
    return out.reshape(x.shape).astype(np.float32)


if __name__ == "__main__":
    xs = np.random.randn(4, 2048, D).astype(np.float32)
    ws = (np.broadcast_to(np.eye(BS, dtype=np.float32), (R, BS, BS))
          + 0.02 * np.random.randn(R, BS, BS).astype(np.float32))
    o = kernel(xs, ws)
    print("kernel ran, out shape", o.shape, o.dtype)


# revision 67
# speedup vs baseline: 1.0029x; 1.0029x over previous
"""Trainium2 Bass kernel for nn_BlockDiagonalLinear_text (hyperbolic block-diag linear).

Math: the reference's per-row operations are all scalar row-scalings, so
  out = alpha_row * y   with  y = x @ blockdiag(W_1..W_16).T
where alpha_row is a chain of tanh/artanh/sqrt scalars of ||x_row|| and
||y_row||.  Both artanh(clip(tanh(t))) compositions in that chain are
min(t, artanh(clip)) up to fp32 roundoff (validated to 3e-4 rel), which
removes two Ln activations and ~10 small ops per chain.

Sharding: data-parallel over rows. 8192 rows -> 8 cores x 1024 rows.
Weights replicated. bf16 on the whole matmul path (l2 tolerance is 2e-2;
bf16 lands ~2e-3): inputs are cast to bf16 on host, output is written
bf16 and cast back to f32 on host, halving all DMA streams.

Each core's x shard is laid out on host in transposed (k-on-partition)
tile order -- xt[tile][p, kc*128+r] = x[tile*128+r, kc*128+p] -- the
layout the PE wants for the stationary operand. This removes all
on-device transposes and their PSUM->SBUF copies. qx = ||x_row||^2 is
computed with a DVE square of xt (2-byte fast path) plus per-k-chunk
matmuls against a ones column (stationary = xsq chunk), accumulating
over the 32 chunks in PSUM; the result lands directly on partitions.

Per-core per 128-row tile:
  DMA xt tile -> DVE xt^2 -> PE qx-matmuls + block matmuls -> Pool
  evacuates y banks (f32 PSUM -> bf16 SBUF) -> ACT y^2 row-sums (qy).
  Every CHAIN_K tiles a batched scalar chain ([128,K] ops on Pool, DVE
  recips, ACT Ln/Exp) produces alpha; chain emission is delayed one
  tile so its serial span never head-of-line-blocks Pool's y-copies.
  DVE scales y (2-byte fast path); outs ship as one batched DMA per
  chain group.

All DMAs ride the SP queue in an order where x-ins carry no waits
(one pool slot per tile) and outs trail, so out-DMAs waiting on scale
results never block the input stream. An explicit LoadActFuncSet pins
the natural_log_exp table (square, ln, exp in one table) so the bacc
table-load pass doesn't thrash tables.
"""
import sys
import numpy as np

for _p in ("/opt/trn_rl_repo", "/root/.axon_site/_ro/trn_rl_repo"):
    if _p not in sys.path:
        sys.path.append(_p)

import ml_dtypes
import concourse.bass as bass
import concourse.bacc as bacc
import concourse.mybir as mybir
from concourse import tile
from concourse.bass_utils import run_bass_kernel_spmd

R, BS = 16, 256           # 16 diagonal blocks of 256x256
D = R * BS                # 4096
P = 128                   # partitions
N_CORES = 8
ROWS_TOTAL = 4 * 2048     # 8192
ROWS_CORE = ROWS_TOTAL // N_CORES   # 1024
NT = ROWS_CORE // P       # 8 tiles of 128 rows per core
CHAIN_K = 1               # tiles per batched scalar chain
WCOLS = 2 * R * BS        # 8192 weight columns (k-major chunks)

f32 = mybir.dt.float32
bf16 = mybir.dt.bfloat16
AF = mybir.ActivationFunctionType
OP = mybir.AluOpType

CLIP_Z = float(np.float32(1.0) - np.float32(1e-5))          # 0.99999
MAXNORM = float(np.float32(1.0 - 1e-3) / np.float32(0.1))   # 9.99
ATC1_X2 = float(2.0 * np.arctanh(np.float64(np.float32(CLIP_Z))))
ATC2 = float(np.arctanh(0.999))

# act_info.json table index of natural_log_exp_and_others (square, ln,
# exp, copy, identity in one table).
NLE_TABLE_ID = 6

XB = D // P               # 32 k-chunks per tile
YB = 512                  # y bank width (f32): [128, 512] = one bank
NBANK_Y = D // YB         # 8 y banks per tile


def build_nc():
    nc = bacc.Bacc()
    xt_d = nc.declare_dram_parameter("xt", [ROWS_CORE, D], bf16, isOutput=False)
    w_d = nc.declare_dram_parameter("w", [P, WCOLS], bf16, isOutput=False)
    out_d = nc.declare_dram_parameter("out", [ROWS_CORE, D], bf16, isOutput=True)

    with tile.TileContext(nc) as tc:
        with (
            tc.tile_pool(name="wpool", bufs=1) as wpool,
            tc.tile_pool(name="xpool", bufs=NT) as xpool,
            tc.tile_pool(name="sqpool", bufs=2) as sqpool,
            tc.tile_pool(name="ypool", bufs=6) as ypool,
            tc.tile_pool(name="opool", bufs=2) as opool,
            tc.tile_pool(name="scrpool", bufs=1) as scrpool,
            tc.tile_pool(name="stats", bufs=2) as stats,
            tc.tile_pool(name="psq", bufs=2, space="PSUM") as psq,
            tc.tile_pool(name="psy", bufs=3, space="PSUM") as psy,
        ):
            # SP DMA order: all 8 xt tiles (no waits - one slot each)
            # interleaved with weight quarters; outs are emitted later so
            # they trail in queue order.
            w_sb = wpool.tile([P, WCOLS], bf16, name="w_sb")
            x_tiles = []
            wq = WCOLS // 4
            for i in range(NT):
                x_sb = xpool.tile([P, D], bf16, tag="x", name=f"x_{i}")
                if i == 0:
                    # first tile split in halves: PE can start on the low
                    # k-chunks ~1.5us earlier
                    nc.sync.dma_start(out=x_sb[:, 0:D // 2],
                                      in_=xt_d[0:P, 0:D // 2])
                    nc.sync.dma_start(out=x_sb[:, D // 2:D],
                                      in_=xt_d[0:P, D // 2:D])
                else:
                    nc.sync.dma_start(out=x_sb[:],
                                      in_=xt_d[i * P:(i + 1) * P, :])
                x_tiles.append(x_sb)
                if i == 0:
                    for q in range(4):
                        nc.sync.dma_start(out=w_sb[:, q * wq:(q + 1) * wq],
                                          in_=w_d[:, q * wq:(q + 1) * wq])
            scratch = scrpool.tile([P, D], bf16, name="scratch")
            ones_full = scrpool.tile([P, 2], bf16, name="ones_full")
            nc.vector.memzero(ones_full[:])
            nc.gpsimd.tensor_scalar_add(ones_full[:], ones_full[:], 1.0)
            ones_sb = ones_full[:, 0:1]

            # Pin the one act table that covers Square/Ln/Exp so the bacc
            # load pass doesn't alternate natural_log <-> exp tables.
            ld = mybir.InstLoadActFuncSet(
                name=nc.get_next_instruction_name(), ins=[], outs=[],
                act_func_set_id=NLE_TABLE_ID)
            nc.scalar.add_instruction(ld)

            def st(shape, tag, dtype=f32):
                return stats.tile(shape, dtype, tag=tag, name=tag)

            V = nc.vector
            G = nc.gpsimd
            K = CHAIN_K

            def emit_tile(i, qq, j):
                """qx + block matmuls + y evacuation + qy for tile i"""
                xt_sb = x_tiles[i]

                # block matmuls: y[:, r*256:+256] = x_blk_r @ W_r.T
                # grouped two blocks per PSUM bank; Pool evacuates each
                # bank to bf16 SBUF
                y_sb = ypool.tile([P, D], bf16, tag="y", name=f"y_{i}")
                for g in range(NBANK_Y // 2):
                    py = psy.tile([P, 2 * YB], f32, tag="py",
                                  name=f"py_{i}_{g}")
                    for rr in range(4):
                        r = 4 * g + rr
                        for c in range(2):
                            kc = 2 * r + c
                            nc.tensor.matmul(
                                py[:, rr * BS:(rr + 1) * BS],
                                xt_sb[:, kc * P:(kc + 1) * P],
                                w_sb[:, kc * BS:(kc + 1) * BS],
                                start=(c == 0), stop=(c == 1),
                            )
                    # PSUM evacuation: GPSIMD can't touch PSUM on real
                    # HW, so split double-bank copies between ACT and DVE
                    dstv = y_sb[:, g * 2 * YB:(g + 1) * 2 * YB]
                    if g % 2 == 0:
                        nc.scalar.activation(dstv, py[:], AF.Copy)
                    else:
                        V.tensor_copy(dstv, py[:])

                # qx = sum_k x^2: DVE squares the transposed tile (2-byte
                # fast path); per-chunk matmuls with stationary = xsq
                # chunk and moving = ones column accumulate the partition
                # sums in PSUM, landing qx per-row on partitions.
                xsq = sqpool.tile([P, D], bf16, tag="xsq", name=f"xsq_{i}")
                V.tensor_tensor(out=xsq[:], in0=xt_sb[:], in1=xt_sb[:],
                                op=OP.mult)
                pqx = psq.tile([P, YB], f32, tag="pqx", name=f"pqx_{i}")
                for kc in range(XB):
                    nc.tensor.matmul(
                        pqx[0:P, 0:1], xsq[:, kc * P:(kc + 1) * P], ones_sb,
                        start=(kc == 0), stop=(kc == XB - 1),
                    )
                V.tensor_copy(qq[:, j:j + 1], pqx[0:P, 0:1])

                # qy = sum_j y^2 (row-wise) on ACT
                nc.scalar.activation(scratch[:], y_sb[:], AF.Square,
                                     accum_out=qq[:, K + j:K + j + 1])
                return y_sb

            def emit_chain(b, qq, y_tiles):
                """batched per-row scalar chain + scale + batched out-DMA.
                artanh(clip(tanh(t))) folds to min(t, artanh(clip))."""
                lnq = st([P, 2 * K], "lnq")
                nc.scalar.activation(lnq[:], qq[:], AF.Ln)
                U = st([P, 2 * K], "U")   # [u | y_n] = exp(0.5 ln q)
                nc.scalar.activation(U[:], lnq[:], AF.Exp, scale=0.5)

                t1 = st([P, K], "t1")     # 0.1 * max(u, 1e-5)
                V.tensor_scalar(out=t1[:], in0=U[:, 0:K], scalar1=1e-5,
                                scalar2=0.1, op0=OP.max, op1=OP.mult)
                r1 = st([P, K], "r1")
                V.reciprocal(r1[:], t1[:])
                d_ = st([P, K], "d_")     # 2*artanh(clip(tanh(t1)))
                V.tensor_scalar(out=d_[:], in0=t1[:], scalar1=2.0,
                                scalar2=ATC1_X2, op0=OP.mult, op1=OP.min)
                yns = st([P, K], "yns")
                V.tensor_scalar_max(yns[:], U[:, K:2 * K], 1e-20)
                w1 = st([P, K], "w1")
                V.tensor_mul(w1[:], U[:, K:2 * K], r1[:])
                w2 = st([P, K], "w2")
                V.tensor_mul(w2[:], w1[:], d_[:])
                argt = st([P, K], "argt")
                V.tensor_scalar(out=argt[:], in0=w2[:], scalar1=0.05,
                                scalar2=15.0, op0=OP.mult, op1=OP.min)
                Et = st([P, K], "Et")
                nc.scalar.activation(Et[:], argt[:], AF.Exp, scale=2.0)
                e2 = st([P, K], "e2")
                V.tensor_scalar_add(e2[:], Et[:], 1.0)
                r3 = st([P, K], "r3")
                V.reciprocal(r3[:], e2[:])
                ttx = st([P, K], "ttx")   # tanh(arg_t)
                V.tensor_scalar(out=ttx[:], in0=r3[:], scalar1=-2.0, scalar2=1.0,
                                op0=OP.mult, op1=OP.add)
                nrm = st([P, K], "nrm")
                V.tensor_scalar(out=nrm[:], in0=ttx[:], scalar1=10.0,
                                scalar2=1e-5, op0=OP.mult, op1=OP.max)
                ryn = st([P, K], "ryn")
                V.reciprocal(ryn[:], yns[:])
                gs = st([P, K], "gs")
                V.tensor_mul(gs[:], ttx[:], ryn[:])
                rn = st([P, K], "rn")
                V.reciprocal(rn[:], nrm[:])
                pf = st([P, K], "pf")     # min(MAXNORM/nrm, 1)
                V.tensor_scalar(out=pf[:], in0=rn[:], scalar1=MAXNORM,
                                scalar2=1.0, op0=OP.mult, op1=OP.min)
                zb = st([P, K], "zb")     # 0.1*min(max(10 ttx,1e-5),MAXNORM)
                V.tensor_scalar(out=zb[:], in0=ttx[:], scalar1=1e-6,
                                scalar2=0.999, op0=OP.max, op1=OP.min)
                db = st([P, K], "db")     # 2*artanh(clip(tanh(argt)))
                V.tensor_scalar(out=db[:], in0=argt[:], scalar1=ATC2,
                                scalar2=2.0, op0=OP.min, op1=OP.mult)
                rzb = st([P, K], "rzb")
                V.reciprocal(rzb[:], zb[:])
                a1 = st([P, K], "a1")
                V.tensor_mul(a1[:], gs[:], pf[:])
                a2 = st([P, K], "a2")
                V.tensor_mul(a2[:], db[:], rzb[:])
                al = st([P, K], "al")
                V.tensor_mul(al[:], a1[:], a2[:])
                mask = st([P, K], "mask")
                V.tensor_scalar(out=mask[:], in0=qq[:, K:2 * K], scalar1=0.0,
                                scalar2=None, op0=OP.is_gt)
                alm = st([P, K], "alm")
                V.tensor_mul(alm[:], al[:], mask[:])

                # out = y * alpha * 5  (5 = 10 from gs x 0.5 from artanh
                # halves); both tiles staged into one buffer, shipped as a
                # single batched DMA trailing on the SP queue
                y_out = opool.tile([P, K * D], bf16, tag="yo", name=f"yo_{b}")
                for j, y_sb in enumerate(y_tiles):
                    V.tensor_scalar(out=y_out[:, j * D:(j + 1) * D],
                                    in0=y_sb[:],
                                    scalar1=alm[:, j:j + 1], scalar2=5.0,
                                    op0=OP.mult, op1=OP.mult)
                i0 = b * K
                dst = out_d[i0 * P:(i0 + K) * P, :].rearrange(
                    "(h p) c -> p h c", p=P)
                nc.sync.dma_start(
                    out=dst, in_=y_out[:].rearrange("p (h c) -> p h c", h=K))

            # main loop: chain emitted right after its batch's tiles
            qq = None
            y_tiles = []
            for i in range(NT):
                b, j = divmod(i, K)
                if j == 0:
                    qq = st([P, 2 * K], "qq")
                    y_tiles = []
                y_tiles.append(emit_tile(i, qq, j))
                if j == K - 1:
                    emit_chain(b, qq, y_tiles)
    nc.finalize()
    return nc


_NC = None


def _get_nc():
    global _NC
    if _NC is None:
        _NC = build_nc()
    return _NC


def _prep_weights(weights: np.ndarray) -> np.ndarray:
    # w_sb[:, (2r+c)*256:+256][p, j] = W[r, j, k=c*128+p]
    wt = (weights.astype(np.float32).transpose(0, 2, 1)      # [r, k, j]
          .reshape(R, 2, P, BS).transpose(2, 0, 1, 3)        # [p, r, c, j]
          .reshape(P, WCOLS))
    return np.ascontiguousarray(wt.astype(ml_dtypes.bfloat16))


def _prep_x(x: np.ndarray) -> np.ndarray:
    # Transposed shard layout: per core, per 128-row tile,
    # xt[p, kc*128 + r] = x[tile*128 + r, kc*128 + p]
    xf = np.asarray(x, dtype=np.float32).reshape(
        N_CORES, NT, P, XB, P)                  # [core, tile, r, kc, p]
    xt = xf.transpose(0, 1, 4, 3, 2)            # [core, tile, p, kc, r]
    return np.ascontiguousarray(xt).astype(
        ml_dtypes.bfloat16).reshape(N_CORES, ROWS_CORE, D)


def kernel(x: np.ndarray, weights: np.ndarray) -> np.ndarray:
    nc = _get_nc()
    xt = _prep_x(x)
    wid = _prep_weights(np.asarray(weights))
    in_maps = [
        {"xt": xt[i], "w": wid}
        for i in range(N_CORES)
    ]
    res = run_bass_kernel_spmd(nc, in_maps, list(range(N_CORES)))
    out = np.concatenate([res.results[i]["out"] for i in range(N_CORES)], axis=0)
    return out.reshape(x.shape).astype(np.float32)


if __name__ == "__main__":
    xs = np.random.randn(4, 2048, D).astype(np.float32)
    ws = (np.broadcast_to(np.eye(BS, dtype=np.float32), (R, BS, BS))
          + 0.02 * np.random.randn(R, BS, BS).astype(np.float32))
    o = kernel(xs, ws)
    print("kernel ran, out shape", o.shape, o.dtype)
